# revision 40
# baseline (speedup 1.0000x reference)
"""AdaptiveConv2d Trainium2 kernel (v2: full-array conv mapping).

Reference computation (B=32, CIN=32, COUT=64, K=3, H=W=128, FIN=64):
    h   = relu(z @ w1.T + b1); h = relu(h @ w2.T + b2)
    aw  = relu(h @ w3.T + b3)                      # (B, 18496)
    kern = aw[:, :18432] -> (B, 64, 32, 3, 3)      # per-sample conv weights
    bias = aw[:, 18432:]                           # (B, 64)
    y = relu(conv2d_same(x, kern) + bias)          # (B, 64, 128, 128)

Strategy: pure data parallel over 8 NeuronCores, 4 samples per core.

Conv mapping (75% PE-cell utilization vs 50% for the block-diag pair
scheme): contraction rows = (dy in 0..4, ci in 0..32) = 128 full rows,
where dy indexes 4 consecutive padded-x rows covering one OUTPUT ROW
PAIR (rows 2rp, 2rp+1); PE columns = (b in {0,1}, cout) = 128 full
cols, b selecting which output row of the pair.  lhsT[(dy,ci),(b,co)]
= w[ky=dy-b, kx, ci, co] (zero if dy-b not in 0..2) is 75% dense.  The
three kx taps accumulate into one PSUM bank via rhs column shifts.
Per sample: 16 groups x 3 kx matmuls of 512 cols = full-array streams;
4 x 16 x 3 = 192 matmuls/core ~= 41us of PE streaming.

x SBUF layout xq[s]: partition (dy*32+ci), free [j, c]: partitions
0:64 hold xpad[ci, 2j+dy] (dy=0,1), partitions 64:128 hold
xpad[ci, 2j+2+(dy-2)] -- the same rows shifted by one j, so the top
half is a duplicate of the bottom half at j+1.  Sample 0 is
host-duplicated (one full-partition load, on the critical path);
samples 1-3 load 64 partitions and duplicate on-device (SBUF->SBUF
DMA) overlapped under the previous sample's conv.

Other moves:
 - output staged and DMA'd in bf16 (halves output HBM traffic; host
   upcasts), one 1MB linear DMA per half-sample
 - ~40 dummy matmuls at kernel start keep the PE HAM clock-gate warm
   (2.4GHz) through the w3-load window so MLP+conv never run at 1.2GHz
 - w3 host-permuted so generated weights come out (ci8, ky, kx, co)
   per cin-chunk; the block lhsT is built in two SBUF->SBUF DMA hops
   (4 chunk-merge DMAs then 8 ky-duplicating DMAs), each balanced to
   <=3 AP dims
 - input DMAs ordered by need on the sync queue (w3, then x chunks);
   small tensors on the scalar queue; dups/outs on gpsimd/scalar

Compute bf16, accumulate f32, output bf16. L2 rel err ~5e-3.
"""

import sys
import types

import numpy as np
import ml_dtypes

BF16 = ml_dtypes.bfloat16

B, CIN, COUT, KS, H, W, FIN = 32, 32, 64, 3, 128, 128, 64
L1, L2 = 20, 30
NKW = CIN * COUT * KS * KS  # 18432
NOUT = NKW + COUT  # 18496
N_CORES = 8
BS = B // N_CORES  # 4 samples per core
NCW = NKW // 4  # 4608 weight columns per cin-chunk
NCH = NCW + COUT  # 4672 including bias tail
XJ = 65  # j extent of xq (row pairs + 1 for the dup source)
XC = 132  # padded col extent (130 used)
N_DUMMY = 8  # PE warm-up matmuls bridging the prologue->w3-arrival gap
N_DUMMY2 = 27  # post-MLP warm-up holding HAM at 2.4GHz until the conv starts


def _install_ntff_hook():
    """Make run_bass_kernel_spmd(trace=True) work under axon by providing
    the antenv.axon_hooks module the image lacks. Safe no-op on failure."""
    try:
        if "antenv.axon_hooks" in sys.modules:
            return
        import antenv

        mod = types.ModuleType("antenv.axon_hooks")
        mod._hook = None
        mod.set_axon_ntff_profile_hook = lambda h: setattr(mod, "_hook", h)
        mod.get_axon_ntff_profile_hook = lambda: mod._hook
        sys.modules["antenv.axon_hooks"] = mod
        antenv.axon_hooks = mod
        from trn_agent_boot.trn_boot import _ntff_profile_via_ctypes

        hook = _ntff_profile_via_ctypes("/opt/axon/libaxon_pjrt.so")
        if hook is not None:
            mod.set_axon_ntff_profile_hook(hook)
    except Exception:
        pass


def _enable_ldw_opt():
    """Flip walrus --enable-ldw-opt to true: dedupes/back-ground-buffers the
    per-matmul LDWEIGHTS so back-to-back matmuls stream at full rate."""
    try:
        from concourse import bass_utils as _bu

        if getattr(_bu.run_command, "_ldw_patched", False):
            return
        _orig = _bu.run_command

        def _patched(cmd, *a, **k):
            try:
                cmd = [
                    ("--enable-ldw-opt=true" if c == "--enable-ldw-opt=false" else c)
                    for c in cmd
                ]
            except TypeError:
                pass
            return _orig(cmd, *a, **k)

        _patched._ldw_patched = True
        _bu.run_command = _patched
    except Exception:
        pass


def build_nc():
    import concourse.tile as tile
    from concourse import bacc, mybir

    dt = mybir.dt
    Relu = mybir.ActivationFunctionType.Relu
    Copy = mybir.ActivationFunctionType.Copy

    nc = bacc.Bacc(
        "TRN2", target_bir_lowering=False, debug=False, num_devices=N_CORES
    )
    # all samples: full 128-partition layout (host-duplicated top half)
    xa0 = nc.dram_tensor("xa0", [128, XJ, XC], dt.bfloat16, kind="ExternalInput")
    xa123 = nc.dram_tensor(
        "xa123", [BS - 1, 128, XJ, XC], dt.bfloat16, kind="ExternalInput"
    )
    zaT = nc.dram_tensor("zaT", [FIN + 1, BS], dt.bfloat16, kind="ExternalInput")
    w1a = nc.dram_tensor("w1a", [FIN + 1, L1], dt.bfloat16, kind="ExternalInput")
    w2a = nc.dram_tensor("w2a", [L1 + 1, L2], dt.bfloat16, kind="ExternalInput")
    # w3 split into 4 column-chunks stacked on partitions: row 32c+k is
    # (w3.T row k) of chunk c for k<30, row 32c+30 is b3 of chunk c,
    # row 32c+31 is zeros.  Chunk c covers ci in [8c, 8c+8); its 4608
    # cols are ordered (ci8, ky, kx, co).  Chunk 3 also carries the 64
    # conv-bias cols at the tail (others zero-padded).
    w3a = nc.dram_tensor("w3a", [128, NCH], dt.bfloat16, kind="ExternalInput")
    # output: [s, hs, (b,co), gg, rp, c] bf16; y[s,co,64*hs+8*gg+2*rp+b,c]
    outd = nc.dram_tensor(
        "out", [BS, 2, 128, 8, 4, W], dt.bfloat16, kind="ExternalOutput"
    )

    with tile.TileContext(nc) as tc:
        with (
            tc.tile_pool(name="const", bufs=1) as cp,
            tc.tile_pool(name="outp", bufs=3) as op,
        ):
            xq = [
                cp.tile([128, XJ, XC], dt.bfloat16, name=f"xq{s}") for s in range(BS)
            ]
            w3s = cp.tile([128, NCH], dt.bfloat16)
            # generated weights: aw[32c+8s, ky*1536+ci8*192+kx*64+co]
            aw = cp.tile([128, NCH], dt.bfloat16)
            # conv lhsT staging: wq[(dy,ci), s, b, kx, co]
            wq = cp.tile([128, BS, 2, KS, COUT], dt.bfloat16)
            # fused lhsT: wqf[(dy,ci), s, kx, (b,co)] -- one matmul per
            # (group, kx) with a single hidden LDWEIGHTS
            wqf = cp.tile([128, BS, KS, 128], dt.bfloat16)
            zs = cp.tile([FIN + 1, BS], dt.bfloat16)
            w1s = cp.tile([FIN + 1, L1], dt.bfloat16)
            w2s = cp.tile([L1 + 1, L2], dt.bfloat16)
            h1a = cp.tile([L1 + 1, BS], dt.bfloat16)
            h2a = cp.tile([128, 32], dt.bfloat16)
            dumT = cp.tile([128, 512], dt.bfloat16)
            # per-sample conv bias, transposed onto partitions: column 8s
            # of btT/biasF holds bias_s[co] at partition b*64+co (b both)
            btT = cp.tile([128, 32], dt.bfloat16)
            biasF = cp.tile([128, 32], dt.float32)

            # ---- input DMAs ----
            # sync ring, ordered by need: w3 in 4 chunks (bias tail first,
            # then one chunk per ky block) so the MLP tail and the weight
            # rearranges pipeline behind the stream; sample 0 in two
            # j-chunks; then each later sample's load immediately followed
            # by its on-device duplicate (same ring = ordered).
            # decoy absorbs the ~5us first-DMA completion latency
            nc.sync.dma_start(dumT[1:2, 0:2], zaT.ap()[0:1, 0:2])
            for c3 in range(KS):
                n0, n1 = c3 * 1536, (c3 + 1) * 1536
                nc.sync.dma_start(w3s[:, n0:n1], w3a.ap()[:, n0:n1])
            nc.sync.dma_start(w3s[:, NCW:NCH], w3a.ap()[:, NCW:NCH])
            JH = 33
            nc.sync.dma_start(xq[0][:, 0:JH, :], xa0.ap()[:, 0:JH, :])
            nc.sync.dma_start(xq[0][:, JH:XJ, :], xa0.ap()[:, JH:XJ, :])
            for s in [2, 1, 3]:
                nc.sync.dma_start(xq[s][:, :, :], xa123.ap()[s - 1])
            # scalar queue: small tensors
            nc.scalar.dma_start(zs[:], zaT.ap())
            nc.scalar.dma_start(w1s[:], w1a.ap())
            nc.scalar.dma_start(w2s[:], w2a.ap())
            # pre-warm the gpsimd SWDGE path
            nc.gpsimd.dma_start(dumT[0:1, 0:1], zaT.ap()[0:1, 0:1])

            # ---- early memsets (MLP inputs first: they gate h1/h2) ----
            nc.vector.memset(h1a[:], 1.0)
            nc.vector.memset(h2a[:], 1.0)
            nc.vector.memset(dumT[:], 0.0)

            # ---- PE warm-up (bridge prologue -> w3 arrival) ----
            with tc.tile_pool(name="dummp", bufs=1, space="PSUM") as dmp:
                dup = dmp.tile([128, 512], dt.float32)
                for i in range(N_DUMMY):
                    nc.tensor.matmul(
                        dup[:], dumT[:, 0:128], dumT[:], start=(i == 0), stop=False
                    )

                # only the never-DMA-written corner blocks need zeroing
                nc.vector.memset(wq[96:128, :, 0, :, :], 0.0)
                nc.vector.memset(wq[0:32, :, 1, :, :], 0.0)

                # ---- MLP generating conv weights ----
                with tc.tile_pool(name="mlpp", bufs=1, space="PSUM") as mp:
                    h1p = mp.tile([L1, BS], dt.float32)
                    nc.tensor.matmul(h1p[:], w1s[:], zs[:], start=True, stop=True)
                    nc.scalar.activation(h1a[0:L1, :], h1p[:], Relu)

                    h2p = mp.tile([L2, BS], dt.float32)
                    nc.tensor.matmul(h2p[:], w2s[:], h1a[:], start=True, stop=True)
                    # h2 replicated into 4 chunk-blocks; rows 32c+30 stay
                    # 1.0 (bias feature), rows 32c+31 are 1.0 x zero w3
                    # row = 0.  sample s lands in column 8*s.
                    for c in range(4):
                        nc.scalar.activation(
                            h2a[32 * c : 32 * c + L2, 0 : 8 * BS : 8],
                            h2p[:],
                            Relu,
                        )

                    # final layer: 4 chunks as concurrent 32-row PE tiles.
                    # The small bias tile (jt=9) runs first (its w3 chunk
                    # arrives first, and its relu on ScalarE alone) so the
                    # per-sample bias DMAs can issue before the rearranges.
                    ntile = (NCH + 511) // 512
                    for jt in range(ntile):
                        n0 = jt * 512
                        n1 = min(NCH, n0 + 512)
                        awp = mp.tile(
                            [128, n1 - n0],
                            dt.float32,
                            tag="awp",
                            bufs=5,
                            name="awp",
                        )
                        for c in range(4):
                            nc.tensor.matmul(
                                awp[32 * c : 32 * c + 32, :],
                                h2a[32 * c : 32 * c + 32, :],
                                w3s[32 * c : 32 * c + 32, n0:n1],
                                start=True,
                                stop=True,
                                tile_position=(32 * c, 32 * c),
                            )
                        if jt == ntile - 1:
                            nc.scalar.activation(
                                aw[:, n0:n1], awp[:], Relu
                            )
                            # bias -> partitions via DVE 32x32 block
                            # transposes (co halves x b halves), then one
                            # f32 convert.  No DMA involved.
                            for bb in range(2):
                                for ch in range(2):
                                    nc.vector.transpose(
                                        btT[64 * bb + 32 * ch : 64 * bb + 32 * ch + 32, :],
                                        aw[96:128, NCW + 32 * ch : NCW + 32 * ch + 32],
                                    )
                            nc.vector.tensor_copy(biasF[:], btT[:])
                            continue
                        mid = (n0 + n1) // 2
                        nc.vector.tensor_scalar_max(
                            aw[:, n0:mid], awp[:, 0 : mid - n0], 0.0
                        )
                        nc.scalar.activation(
                            aw[:, mid:n1], awp[:, mid - n0 : n1 - n0], Relu
                        )

                # ---- second warm-up block: keep the PE busy (HAM warm)
                # through the relu/rearrange window until the conv starts
                for i in range(N_DUMMY2):
                    nc.tensor.matmul(
                        dup[:], dumT[:, 0:128], dumT[:], start=False, stop=(i == N_DUMMY2 - 1)
                    )

            # ---- rearrange generated weights into the block lhsT ----
            # per (s, b, ky): copy one 32-partition block (all ci) of one
            # ky tap into wq partitions 32*(b+ky).  Both sides are plain
            # partition-outermost APs.  Sample 0's six go on ScalarE (its
            # DMA window is otherwise free until the epilogues start);
            # later samples issue from gpsimd, sample-major.
            for s in [0, 2, 1, 3]:
                for b in range(2):
                    for ky in range(KS):
                        eng = nc.scalar if s != 3 else nc.gpsimd
                        p0 = 32 * (b + ky)
                        n0 = ky * 1536
                        src = aw[:, n0 : n0 + 1536].rearrange(
                            "(c ss) (ci8 r) -> c ss ci8 r", c=4, ci8=8
                        )
                        eng.dma_start(
                            wq[p0 : p0 + 32, s, b, :, :],
                            src[:, 8 * s, :, :],
                            single_packet=True,
                        )
                # hop 2: interleave (b, co) contiguously per kx
                for kx in range(KS):
                    eng = nc.scalar if s != 3 else nc.gpsimd
                    eng.dma_start(
                        wqf[:, s, kx, :], wq[:, s, :, kx, :], single_packet=True
                    )

            # ---- conv: 4 samples x 2 half-samples x (3 kx x 8 banks) ----
            with tc.tile_pool(name="cps", bufs=8, space="PSUM") as cps:
                for si, s in enumerate([0, 2, 1, 3]):
                    for hs in range(2):
                        pss = [
                            cps.tile([128, 4, W], dt.float32, tag="ps", name="ps")
                            for _ in range(8)
                        ]
                        for kx in range(KS):
                            for gg in range(8):
                                j0 = (hs * 8 + gg) * 4
                                nc.tensor.matmul(
                                    pss[gg][:],
                                    wqf[:, s, kx, :],
                                    xq[s][:, j0 : j0 + 4, kx : kx + W],
                                    start=(kx == 0),
                                    stop=(kx == KS - 1),
                                )
                        obig = op.tile(
                            [128, 8, 4, W], dt.bfloat16, tag="ob", name="ob"
                        )
                        # sample 0's epilogues run entirely on VectorE:
                        # ScalarE is still issuing the later samples'
                        # rearrange DMAs in that window.
                        for gg in range(8):
                            if si > 1 and gg % 2 == 0:
                                nc.scalar.activation(
                                    obig[:, gg],
                                    pss[gg][:],
                                    Relu,
                                    bias=biasF[:, 8 * s : 8 * s + 1],
                                )
                            else:
                                nc.vector.tensor_scalar(
                                    obig[:, gg],
                                    pss[gg][:],
                                    biasF[:, 8 * s : 8 * s + 1],
                                    0.0,
                                    mybir.AluOpType.add,
                                    mybir.AluOpType.max,
                                )
                        hsid = si * 2 + hs
                        if hsid < 2 * BS - 1:
                            eng = nc.scalar if hsid % 2 == 0 else nc.gpsimd
                            eng.dma_start(outd.ap()[s, hs], obig[:])
                        else:
                            # flush the final half-sample in quarters so
                            # earlier quarters drain while later compute
                            for q in range(2):
                                for e, eng in enumerate((nc.scalar, nc.gpsimd)):
                                    gq = q * 4 + e * 2
                                    eng.dma_start(
                                        outd.ap()[s, hs, :, gq : gq + 2],
                                        obig[:, gq : gq + 2],
                                    )

    nc.compile()
    return nc


def _host_prep(x, z, w1, b1, w2, b2, w3, b3):
    """Build per-core input maps (np arrays, bf16 where device expects)."""
    x = np.asarray(x, np.float32)
    z = np.asarray(z, np.float32)
    w1 = np.asarray(w1, np.float32)
    b1 = np.asarray(b1, np.float32)
    w2 = np.asarray(w2, np.float32)
    b2 = np.asarray(b2, np.float32)
    w3 = np.asarray(w3, np.float32)
    b3 = np.asarray(b3, np.float32)

    w1a = np.concatenate([w1.T, b1[None, :]], axis=0).astype(BF16)  # (65, 20)
    w2a = np.concatenate([w2.T, b2[None, :]], axis=0).astype(BF16)  # (21, 30)

    # w3 rows for chunk c, ordered (ky, ci8, kx, co):
    # old j = ((co*CIN + ci)*3 + ky)*3 + kx with ci = 8c + ci8
    ky = np.arange(KS)[:, None, None, None]
    ci8 = np.arange(8)[None, :, None, None]
    kx = np.arange(KS)[None, None, :, None]
    co = np.arange(COUT)[None, None, None, :]
    w3flat = np.concatenate([w3.T, b3[None, :]], axis=0)  # (L2+1, NOUT)
    w3a = np.zeros((128, NCH), np.float32)
    for c in range(4):
        oldj = (
            (co * CIN + (8 * c + ci8)) * KS * KS + ky * KS + kx
        ).reshape(-1)
        w3a[32 * c : 32 * c + L2 + 1, 0:NCW] = w3flat[:, oldj]
    w3a[96 : 96 + L2 + 1, NCW:NCH] = w3flat[:, NKW:NOUT]
    w3a = w3a.astype(BF16)

    HP = H + 2
    in_maps = []
    for core in range(N_CORES):
        sl = slice(core * BS, (core + 1) * BS)
        xs = x[sl].astype(BF16)  # (BS, CIN, H, W)
        xpad = np.zeros((BS, CIN, HP, XC), BF16)
        xpad[:, :, 1 : H + 1, 1 : W + 1] = xs
        # bottom half: partition (dy,ci) dy in {0,1}: rows 2j+dy
        xqb = np.zeros((BS, 64, XJ, XC), BF16)
        for dy in range(2):
            xqb[:, dy * 32 : dy * 32 + 32, :, :] = xpad[
                :, :, dy : dy + 2 * XJ : 2, :
            ]
        # host-duplicated: top half = bottom half at j+1 (all samples)
        xqf = np.zeros((BS, 128, XJ, XC), BF16)
        xqf[:, 0:64] = xqb
        xqf[:, 64:128, 0 : XJ - 1] = xqb[:, :, 1:XJ]
        zaT = np.concatenate(
            [z[sl].T, np.ones((1, BS), np.float32)], axis=0
        ).astype(BF16)  # (65, BS)
        in_maps.append(
            {
                "xa0": xqf[0],
                "xa123": xqf[1:],
                "zaT": zaT,
                "w1a": w1a,
                "w2a": w2a,
                "w3a": w3a,
            }
        )
    return in_maps


_NC_CACHE = {}
LAST_EXEC_NS = None
LAST_TRACE_DIR = None


def _get_nc():
    if "nc" not in _NC_CACHE:
        _NC_CACHE["nc"] = build_nc()
    return _NC_CACHE["nc"]


def kernel(x, z, w1, b1, w2, b2, w3, b3, _trace=False):
    global LAST_EXEC_NS, LAST_TRACE_DIR
    _install_ntff_hook()
    from concourse.bass_utils import run_bass_kernel_spmd

    nc = _get_nc()
    in_maps = _host_prep(x, z, w1, b1, w2, b2, w3, b3)
    kwargs = {}
    if _trace:
        import tempfile

        LAST_TRACE_DIR = tempfile.mkdtemp(prefix="adaptconv_trace_")
        kwargs = dict(trace=True, tmpdir=LAST_TRACE_DIR)
    res = run_bass_kernel_spmd(
        nc, in_maps, core_ids=list(range(N_CORES)), **kwargs
    )
    LAST_EXEC_NS = res.exec_time_ns
    cores = []
    for i in range(N_CORES):
        arr = np.asarray(res.results[i]["out"])  # (BS, 2, 128, 8, 4, W) bf16
        y = (
            arr.reshape(BS, 2, 2, COUT, 8, 4, W)
            .transpose(0, 3, 1, 4, 5, 2, 6)  # s, co, hs, gg, rp, b, c
            .reshape(BS, COUT, H, W)
        )
        cores.append(y)
    return np.concatenate(cores, axis=0).astype(np.float32)
